# revision 104
# baseline (speedup 1.0000x reference)
"""Trainium2 Bass kernel for nn_MDSFF (deformable-sampling sparse attention).

Math (same restructuring as v1, new layouts/engines):
  - tanh-bounded offsets => bilinear sample == 9-tap stencil, weights
    w_d = relu(1 - |clip(t,lo,hi) - d|) with edge masks; 1x1 convs commute
    with shifts so A = k_w@x_aux and Ao = out_w@x_aux are projected once.
  - sim[k,h] = sum_t TW_t[k] * S_t[h],  S_t[h] = sum_{c in h} q_c * A_t,c.
  - final = sum_t G_t * Ao_t,  G_t = sum_j 0.25 * TW_t[j] * attn[j].

v2 speed structure:
  - fp16 everywhere (rel-err budget 2e-2; measured ~1e-3): DVE TensorTensor
    runs 2x on 2-byte dtypes, tensor_scalar 4x; DMA bytes halve.
  - per-(k,h,pixel) maps in pixel-grouped [128 = 32g+j, 512] layout (g =
    8-row band) -- engine cost is per-column, so 4x fewer columns than the
    v1 [32, 2048] layout.
  - conv: 3 dy-passes accumulate in one PSUM region (dy shift folded into
    the rhs view); only the cheap dx-fold (2 adds) is elementwise.
  - fin accumulated in SBUF fp16 by a DVE/Pool add tree (PSUM has no room).
  - explicit engine assignment tables to balance DVE / Pool / Act.

Sharding: 8 cores = 4 samples x 2 H-halves; each core does its 32 rows in
one full frame (2048 px).
"""

import sys

sys.path.insert(0, "/opt/trn_rl_repo")

import numpy as np

import concourse.bass as bass
import concourse.mybir as mybir
from concourse import tile
from concourse.bass_utils import run_bass_kernel_spmd

# ---------------- problem constants (hardcoded per contract) ----------------
B, C, H, W = 4, 256, 64, 64
K = 8
NCORES = 8
ROWS = 32          # center rows per core
HR = 34            # haloed rows per core
XM_W = 2 + 66 * HR     # padded x_main: col = 2 + 66*rh + w (w-halo via pad)
XA_W = 64 * HR         # x_aux: col = 64*rh + w
AW = 2 + 64 * HR       # A/Ao: col = 1 + 64*rh + w, pad cols 0 and AW-1
N1 = 1024          # px per half-frame
NPX = 2048         # px per core
TAPS = [(dy, dx) for dy in (-1, 0, 1) for dx in (-1, 0, 1)]

F32 = mybir.dt.float32
F16 = mybir.dt.float16
AF = mybir.ActivationFunctionType
OP = mybir.AluOpType

# engine assignment knobs (tuned against the cost model)
M_ENG = ["V", "P", "V", "P", "V", "P", "V", "P", "V"]      # M_t = q*A_t
FV_MODE = ["cA", "dP", "cA", "dP", "dP", "cA", "dP", "dP", "cA"]
#   cV: Pool copies Gb psum->fp16 + DVE mult; cA: Act copy + DVE mult;
#   dP: Pool mult direct from psum
FORDER = [0, 2, 5, 8, 1, 6, 3, 4, 7]   # F-loop tap order
UB = 6             # unified one-bank psum tiles (+2 banks: sim tag)

_CACHE = {}


# ============================ program builder ===============================

def _build_program():
    MAX_WAITS = 1

    SPLIT_OK = {
        "InstDrain", "InstNoOp", "InstMatmult", "InstLdweights",
        "InstTensorTensor", "InstActivation", "InstTensorScalarPtr",
        "InstTensorReduce", "InstCopy", "InstMemSet", "InstMemset",
        "InstReciprocal", "InstTensorTensorReduce", "InstTensorCopy",
        "InstDMACopy",
    }

    def split_waits(nc):
        # walrus rejects >MAX_WAITS semaphore waits per instruction; spill
        # extras onto same-engine nops (engine-FIFO instructions only).
        # DMACopy is included ONLY for the tail y-DMAs: they are the last
        # SP instructions, so stalling the SP sequencer on the spilled wait
        # cannot deadlock (no later SP-pushed DMAs for producers to need).
        f = nc.m.functions[0]
        for bb in f.blocks:
            insts = bb.instructions
            out = []
            changed = False
            for inst in insts:
                si = inst.sync_info
                waits = list(si.on_wait) if si and si.on_wait else []
                if (len(waits) > MAX_WAITS
                        and type(inst).__name__ in SPLIT_OK
                        and all(w.wait_reg is None for w in waits)):
                    changed = True
                    rest, keep = waits[:-MAX_WAITS], waits[-MAX_WAITS:]
                    for i in range(0, len(rest), MAX_WAITS):
                        nop = mybir.InstNoOp(
                            name=f"Wspill_{inst.name}_{i}", ins=[], outs=[])
                        nop.engine = inst.engine
                        nop.sync_info = mybir.SyncInfo(
                            on_wait=rest[i : i + MAX_WAITS], on_update=[])
                        nc.register_instruction(nop)
                        out.append(nop)
                    inst.sync_info = mybir.SyncInfo(
                        on_wait=keep, on_update=list(si.on_update or [])
                    )
                out.append(inst)
            if changed:
                bb.instructions = out

    nc = bass.Bass("TRN2", target_bir_lowering=False, debug=False,
                   num_devices=NCORES)

    dp = nc.dram_tensor
    xm_d = dp("xm", [128, 2, XM_W], F16, kind="ExternalInput")
    xa_d = dp("xa", [128, 2, XA_W], F16, kind="ExternalInput")
    # packed weights: qwT(512) kwT(512) owT(512) cwTd(480) selS(64)
    #                 sel16x(32) sel16y(32) i128(128) hs128(128) qb(512)
    #                 sel48(16)
    wcat_d = dp("wcat", [128, 2960], F16, kind="ExternalInput")
    # packed consts: lox(64) hix(64) mxp(64) negmxm(64)
    #                loy(512) hiy(512) myp(512) negmym(512) bperm(1)
    mcat_d = dp("mcat", [128, 3841], F16, kind="ExternalInput")
    y_d = dp("y", [128, 2, NPX], F16, kind="ExternalOutput")

    V = nc.vector
    P = nc.gpsimd
    A_ = nc.scalar

    def mm(out, lhsT, rhs, start, stop, tp=None):
        nc.tensor.matmul(out=out, lhsT=lhsT, rhs=rhs,
                         start=start, stop=stop, skip_group_check=True,
                         tile_position=tp)

    with tile.TileContext(nc) as tc:
        with (
            nc.allow_low_precision(reason="fp16 data: error budget 2e-2, "
                                   "fp16 chain measured ~1e-3"),
            tc.tile_pool(name="pw", bufs=1) as pw,       # weights/consts
            tc.tile_pool(name="pio", bufs=1) as pio,     # xm, xa, y
            tc.tile_pool(name="pbig", bufs=1) as pbig,   # q, A, Ao, off
            tc.tile_pool(name="pmap", bufs=1) as pmap,   # [128,512] maps
            tc.tile_pool(name="pm", bufs=1) as pm,       # M / Fv / acc
            tc.tile_pool(name="psum", bufs=1, space="PSUM") as psp,
        ):
            xm = pio.tile([128, 2, XM_W], F16, tag="xm")
            xa = pio.tile([128, 2, XA_W], F16, tag="xa")
            wcat = pw.tile([128, 2960], F16, tag="wcat")
            mcat = pw.tile([128, 3841], F16, tag="mcat")
            # all input DMAs on SP (engine-issued DMA time counts as
            # that engine's busy); conv weights slice first so the conv
            # can start at ~2us.
            nc.gpsimd.dma_start(out=wcat[:, 1536:2016],
                                in_=wcat_d[:, 1536:2016])
            for cb in range(2):
                nc.sync.dma_start(out=xm[:, cb, :], in_=xm_d[:, cb, :])
            nc.gpsimd.dma_start(out=mcat[:], in_=mcat_d[:])
            for cb in range(2):
                nc.gpsimd.dma_start(out=xa[:, cb, :], in_=xa_d[:, cb, :])
            nc.sync.dma_start(out=wcat[:, 0:1536], in_=wcat_d[:, 0:1536])
            nc.sync.dma_start(out=wcat[:, 2016:2960],
                              in_=wcat_d[:, 2016:2960])

            def w4(o):  # [128, cb, ob, 128] block at col o
                return wcat[:, o : o + 512].rearrange(
                    "p (cb ob m) -> p cb ob m", cb=2, ob=2)

            qwT, kwT, owT = w4(0), w4(512), w4(1024)
            cwTd = wcat[:, 1536:2016].rearrange(
                "p (cb dy m) -> p cb dy m", cb=2, dy=3)     # m = 32*dxi+o
            selS = wcat[:, 2016:2080].rearrange("p (cb j) -> p cb j", cb=2)
            sel16x = wcat[0:16, 2080:2112]
            sel16y = wcat[0:16, 2112:2144]
            i128 = wcat[:, 2144:2272]
            hs128 = wcat[:, 2272:2400]
            qbg = wcat[:, 2400:2912].rearrange("p (g m) -> p g m", g=4)
            sel48 = wcat[0:80, 2912:2960].rearrange(
                "p (d m) -> p d m", d=3)

            lox = mcat[:, 0:64]
            hix = mcat[:, 64:128]
            mxp = mcat[:, 128:192]
            negmxm = mcat[:, 192:256]
            loy = mcat[:, 256:768]
            hiy = mcat[:, 768:1280]
            myp = mcat[:, 1280:1792]
            negmym = mcat[:, 1792:2304]
            bperm = mcat[0:16, 2304:2305]
            mxp512 = mcat[:, 2305:2817]
            mxm512 = mcat[:, 2817:3329]
            mym512 = mcat[:, 3329:3841]

            def xb(t):  # broadcast [128,64] x-const over 8 rows -> [128,8,64]
                return t[:, None, :].broadcast_to([128, 8, 64])

            # ---------------- offset conv (3 dy-passes) + fold + tanh -------
            # D[16*dxi+o, 64*r + wc] = sum_{c,dy} w[o,c,dy,dxi]
            #                                    * xm[c, r+dy, wc-1]
            # rhs flat view: 66-strided rows, start col = 2+66*(1+r0+dy)-1
            # D[16*dxi+o, 66*rr + wc] = sum_{c,dy} w[o,c,dy,dxi]
            #   * xm[c, r0+rr+dy, wc-1];  off tap dxi reads wc = w + dxi.
            off = pbig.tile([16, NPX], F16, tag="off")
            dsbs = {}

            def conv_q(qt):
                r0 = 8 * qt
                dsb = pm.tile([80, 528], F16, tag="dsb", bufs=3,
                              name=f"dsb{qt}")
                for c0, sz in ((0, 512), (512, 16)):
                    dps = psp.tile([128, 512], F32, tag="u", bufs=UB,
                                   name=f"dps{qt}{c0}")
                    nmm = 0
                    for dyi in range(3):
                        for cb in range(2):
                            base = 2 + 66 * (r0 + dyi) - 1
                            mm(dps[0:80, 0:sz],
                               cwTd[:, cb, dyi, :],
                               xm[:, cb, base + c0 : base + c0 + sz],
                               start=(nmm == 0), stop=(nmm == 5))
                            nmm += 1
                    V.tensor_scalar(out=dsb[:, c0 : c0 + sz],
                                    in0=dps[0:80, 0:sz], scalar1=1.0,
                                    scalar2=None, op0=OP.mult)
                dsbs[qt] = dsb

            def fold_q(qt):
                dv = dsbs[qt].rearrange("p (r wc) -> p r wc", wc=66)
                fps = psp.tile([128, 512], F32, tag="u", bufs=UB,
                               name=f"fold{qt}")
                for dxi in range(3):
                    mm(fps[0:16, :], sel48[:, dxi, :],
                       dv[:, 0:8, dxi : dxi + 64],
                       start=(dxi == 0), stop=(dxi == 2))
                A_.activation(out=off[:, 512 * qt : 512 * (qt + 1)],
                              in_=fps[0:16, :], func=AF.Tanh, bias=bperm,
                              scale=1.0)

            conv_q(0); conv_q(1); fold_q(0); conv_q(2)
            fold_q(1); conv_q(3); fold_q(2); fold_q(3)
            # (q-proj mms interleave naturally below; PE stays fed)

            # ---------------- q / A / Ao projections ----------------
            # ---------------- tap-weight maps (grouped [128,512]) ----------
            # partition p = 32g+j (j = 4k+h); col = 64*(r%8)+w, g = r//8.
            # Scheduled right after the conv so the DVE/Pool chains overlap
            # the q/A projections on PE; x-chain on DVE, y-chain on Pool.
            txp = psp.tile([128, 512], F32, tag="u", bufs=UB, name="txp")
            typ = psp.tile([128, 512], F32, tag="u", bufs=UB, name="typ")
            for g in range(4):
                mm(txp[32 * g : 32 * g + 32, :], sel16x,
                   off[:, 512 * g : 512 * (g + 1)], start=True, stop=True,
                   tp=(0, 32 * g))
            for g in range(4):
                mm(typ[32 * g : 32 * g + 32, :], sel16y,
                   off[:, 512 * g : 512 * (g + 1)], start=True, stop=True,
                   tp=(0, 32 * g))

            def gv(t):  # [128, 512] -> [128, 8, 64]
                return t.rearrange("p (r w) -> p r w", w=64)

            tcx = pmap.tile([128, 512], F16, tag="mw", bufs=4, name="tcx")
            V.tensor_tensor(out=gv(tcx), in0=gv(txp), in1=xb(lox), op=OP.max)
            V.tensor_tensor(out=gv(tcx), in0=gv(tcx), in1=xb(hix), op=OP.min)
            tcy = pmap.tile([128, 512], F16, tag="mw", bufs=4, name="tcy")
            V.tensor_tensor(out=tcy[:], in0=typ[:], in1=loy, op=OP.max)
            V.tensor_tensor(out=tcy[:], in0=tcy[:], in1=hiy, op=OP.min)

            # weight maps on DVE/Act only: Pool's software TensorTensor
            # implements just mult/add, and cannot touch PSUM.
            wx, wy = {}, {}
            for ax, (tc_, mkp, mkm) in enumerate(
                    ((tcx, mxp512, mxm512), (tcy, myp, mym512))):
                wd = {}
                for d in (1, -1, 0):
                    wd[d] = pmap.tile([128, 512], F16, tag=f"w{ax}{d}",
                                      name=f"w{ax}_{d}")
                V.scalar_tensor_tensor(out=wd[1][:], in0=tc_[:], scalar=0.0,
                                       in1=mkp, op0=OP.max, op1=OP.mult)
                A_.activation(out=wd[-1][:], in_=tc_[:], func=AF.Relu,
                              scale=-1.0)
                P.tensor_tensor(out=wd[-1][:], in0=wd[-1][:], in1=mkm,
                                op=OP.mult)
                A_.activation(out=wd[0][:], in_=tc_[:], func=AF.Abs)
                V.tensor_scalar(out=wd[0][:], in0=wd[0][:], scalar1=-1.0,
                                scalar2=1.0, op0=OP.mult, op1=OP.add)
                (wx, wy)[ax].update(wd)

            TW = []
            for t, (dy, dx) in enumerate(TAPS):
                tw = pmap.tile([128, 512], F16, tag=f"tw{t}", name=f"TW{t}")
                P.tensor_tensor(out=tw[:], in0=wy[dy][:], in1=wx[dx][:],
                                op=OP.mult)
                TW.append(tw)

            # ---------------- q / A projections ----------------
            # PSUM->SBUF copies rotate across Act/Pool/DVE so no single
            # engine serializes the A assembly (M_t needs the whole of A).
            cpcnt = [0]

            def copy_ps(dst_view, ps_view):
                i = cpcnt[0] % 3
                cpcnt[0] += 1
                if i == 1:
                    V.tensor_scalar(out=dst_view, in0=ps_view, scalar1=1.0,
                                    scalar2=None, op0=OP.mult)
                else:
                    A_.activation(out=dst_view, in_=ps_view, func=AF.Copy)

            q = pbig.tile([128, 2, NPX], F16, tag="q")
            AT = pbig.tile([128, 2, AW], F16, tag="A")
            AoT = pbig.tile([128, 2, AW], F16, tag="Ao")
            for dst in (AT, AoT):
                V.memset(dst[:, :, 0:1], 0.0)
                V.memset(dst[:, :, AW - 1 : AW], 0.0)

            def q_chunk(ob, hf, c0):
                ps = psp.tile([128, 512], F32, tag="u", bufs=UB,
                              name=f"psq{ob}{hf}{c0}")
                for cb in range(2):
                    mm(ps[:], qwT[:, cb, ob, :],
                       xm_rows(xm, cb, 16 * hf + c0 // 64),
                       start=(cb == 0), stop=(cb == 1))
                copy_ps(q[:, ob, N1 * hf + c0 : N1 * hf + c0 + 512], ps[:])

            def proj_chunk(dst, wT, ob, p0, di):
                sz = min(512, 2176 - p0)
                ps = psp.tile([128, 512], F32, tag="u", bufs=UB,
                              name=f"psP{di}{ob}{p0}")
                for cb in range(2):
                    mm(ps[:, 0:sz], wT[:, cb, ob, :],
                       xa[:, cb, p0 : p0 + sz],
                       start=(cb == 0), stop=(cb == 1))
                copy_ps(dst[:, ob, 1 + p0 : 1 + p0 + sz], ps[:, 0:sz])

            # chunks needed by M's first half (q cols 0:1024, A cols
            # 0:~1090) are emitted first so the S-loop can start early
            mixed = []
            for ob in range(2):
                for c0 in (0, 512):
                    mixed.append(("q", ob, c0))
            for p0 in (0, 512, 1024):
                for ob in range(2):
                    mixed.append(("A", ob, p0))
            for ob in range(2):
                for c0 in (0, 512):
                    mixed.append(("q", ob, 1024 + c0))
            for p0 in (1536, 2048):
                for ob in range(2):
                    mixed.append(("A", ob, p0))
            for kind, ob, p0 in mixed:
                if kind == "q":
                    q_chunk(ob, p0 // 1024, p0 % 1024)
                else:
                    proj_chunk(AT, kwT, ob, p0, 0)
            # Ao chunks are interleaved into the S-loop below: they fill PE
            # gaps while M tiles are being produced, and Ao is only needed
            # by the F phase.
            ao_chunks = [(ob, p0) for ob in range(2)
                         for p0 in range(0, 2176, 512)]

            # ---------------- S maps + sim + softmax ----------------
            # sim accumulated on DVE (fp32 SBUF adds) to keep the sim chain
            # off the PE, which is the S-loop bottleneck.
            sim_sb = pmap.tile([128, 512], F16, tag="simb", name="simb")
            # NOTE: "sim" psum tag tiles are reused as fin1 in the F phase
            Ps = []
            for t, (dy, dx) in enumerate(TAPS):
                o_t = 65 + 64 * dy + dx
                M = pm.tile([128, 2, NPX], F16, tag="M", bufs=4,
                            name=f"M{t}")
                eng = V if M_ENG[t] == "V" else P
                nmh = 4 if t == 0 else 2
                for mh in range(nmh):
                    w = NPX // nmh
                    me = (V, P)[mh % 2] if t == 0 else eng
                    me.tensor_tensor(
                        out=M[:, :, w * mh : w * (mh + 1)],
                        in0=q[:, :, w * mh : w * (mh + 1)],
                        in1=AT[:, :, o_t + w * mh : o_t + w * mh + w],
                        op=OP.mult)
                s_ps = psp.tile([128, 512], F32, tag="u", bufs=UB,
                                name=f"sps{t}")
                for g in range(4):
                    for cb in range(2):
                        mm(s_ps[32 * g : 32 * g + 32, :], selS[:, cb, :],
                           M[:, cb, 512 * g : 512 * (g + 1)],
                           start=(cb == 0), stop=(cb == 1), tp=(0, 32 * g))
                P_t = pmap.tile([128, 512], F16, tag="sp", bufs=3,
                                name=f"P{t}")
                V.tensor_tensor(out=P_t[:], in0=s_ps[:], in1=TW[t][:],
                                op=OP.mult)
                if t == 1:
                    P.tensor_tensor(out=sim_sb[:], in0=Ps[0][:], in1=P_t[:],
                                    op=OP.add)
                elif t > 1:
                    P.tensor_tensor(out=sim_sb[:], in0=sim_sb[:], in1=P_t[:],
                                    op=OP.add)
                Ps.append(P_t if t == 0 else None)
                while ao_chunks and len(ao_chunks) > max(4, 10 - 2 * (t + 1)):
                    ob, p0 = ao_chunks.pop(0)
                    proj_chunk(AoT, owT, ob, p0, 1)

            for ob, p0 in ao_chunks:
                proj_chunk(AoT, owT, ob, p0, 1)
            ao_chunks = []

            E = pmap.tile([128, 512], F16, tag="smE", name="E")
            A_.activation(out=E[:], in_=sim_sb[:], func=AF.Exp,
                          bias=0.0, scale=0.125)
            d_ps = psp.tile([128, 512], F32, tag="u", bufs=UB, name="dps")
            mm(d_ps[:], hs128, E[:], start=True, stop=True)
            Rr = pmap.tile([128, 512], F16, tag="smR", name="R")
            V.reciprocal(out=Rr[:], in_=d_ps[:])
            Ff = pmap.tile([128, 512], F16, tag="smF", name="F")
            V.tensor_tensor(out=Ff[:], in0=E[:], in1=Rr[:], op=OP.mult)

            # ---------------- final combine ----------------
            # Q_t is half-independent: compute the 9 maps once.
            Qs = []
            for t in range(9):
                Q_t = pmap.tile([128, 512], F16, tag="qg", bufs=9,
                                name=f"Q{t}")
                P.tensor_tensor(out=Q_t[:], in0=TW[t][:], in1=Ff[:],
                                op=OP.mult)
                Qs.append(Q_t)

            y_sb = pio.tile([128, 2, NPX], F16, tag="y")
            # Both half-frames interleaved per tap (independent chains hide
            # per-hop latency; only hf0-ob0 pins psum).  Accumulation:
            # hf0-ob0 on PE psum, hf0-ob1 and hf1(both ob) as running
            # in-place adds paced by Fv arrival (tail depth = 1 add).
            FV_MODES = ["DV", "AP", "AV", "AP", "AV", "AP", "AV", "AP", "DV"]
            ACC_ENG = [V, P, V, P, V, P, V, V]
            fin0 = [psp.tile([128, 512], F32, tag="u", bufs=UB,
                             name=f"fin0{gl}") for gl in range(2)]
            fin1 = [psp.tile([128, 512], F32, tag="sim", bufs=2,
                             name=f"fin1{gl}") for gl in range(2)]
            acc0 = pm.tile([128, N1], F16, tag="acc", bufs=2, name="acc0")
            acc1 = pm.tile([128, N1], F16, tag="acc", bufs=2,
                           name="acc1")
            prev = [None, None]
            for i, t in enumerate(FORDER):
                dy, dx = TAPS[t]
                for hf in range(2):
                    o_t = 65 + 64 * dy + dx + N1 * hf
                    Fv = pm.tile([128, 2, N1], F16, tag="Fv", bufs=5,
                                 name=f"Fv{hf}{t}")
                    for gl in range(2):
                        g = 2 * hf + gl
                        gb_ps = psp.tile([128, 512], F32, tag="u", bufs=UB,
                                         name=f"gb{hf}{t}{gl}")
                        mm(gb_ps[:], qbg[:, g, :], Qs[t][:],
                           start=True, stop=True)
                        ov = o_t + 512 * gl
                        md = FV_MODES[i]
                        fvv = Fv[:, :, 512 * gl : 512 * (gl + 1)]
                        if md == "DV":
                            V.tensor_tensor(
                                out=fvv,
                                in0=gb_ps[:, None, :].broadcast_to(
                                    [128, 2, 512]),
                                in1=AoT[:, :, ov : ov + 512], op=OP.mult)
                        else:
                            gsb = pm.tile([128, 512], F16, tag="gsb",
                                          bufs=4, name=f"gsb{hf}{t}{gl}")
                            if md == "VP":
                                V.tensor_scalar(out=gsb[:], in0=gb_ps[:],
                                                scalar1=1.0, scalar2=None,
                                                op0=OP.mult)
                            else:
                                A_.activation(out=gsb[:], in_=gb_ps[:],
                                              func=AF.Copy)
                            (P if md in ("AP", "VP") else V).tensor_tensor(
                                out=fvv,
                                in0=gsb[:, None, :].broadcast_to(
                                    [128, 2, 512]),
                                in1=AoT[:, :, ov : ov + 512], op=OP.mult)
                    if hf == 0:
                        for gl in range(2):
                            mm(fin0[gl][:], i128,
                               Fv[:, 0, 512 * gl : 512 * (gl + 1)],
                               start=(i == 0), stop=(i == 8))
                        # ob1 running accumulation
                        if i == 0:
                            prev[0] = Fv
                        elif i == 1:
                            V.tensor_tensor(out=acc0[:],
                                            in0=prev[0][:, 1, :],
                                            in1=Fv[:, 1, :], op=OP.add)
                        elif i < 8:
                            ACC_ENG[i - 1].tensor_tensor(
                                out=acc0[:], in0=acc0[:], in1=Fv[:, 1, :],
                                op=OP.add)
                        else:
                            V.tensor_tensor(out=y_sb[:, 1, 0:N1],
                                            in0=acc0[:], in1=Fv[:, 1, :],
                                            op=OP.add)
                    else:
                        for gl in range(2):
                            mm(fin1[gl][:], i128,
                               Fv[:, 0, 512 * gl : 512 * (gl + 1)],
                               start=(i == 0), stop=(i == 8))
                        if i == 0:
                            prev[1] = Fv
                        elif i == 1:
                            V.tensor_tensor(out=acc1[:],
                                            in0=prev[1][:, 1, :],
                                            in1=Fv[:, 1, :], op=OP.add)
                        elif i < 8:
                            ACC_ENG[8 - i].tensor_tensor(
                                out=acc1[:], in0=acc1[:], in1=Fv[:, 1, :],
                                op=OP.add)
                        else:
                            V.tensor_tensor(out=y_sb[:, 1, N1 : 2 * N1],
                                            in0=acc1[:], in1=Fv[:, 1, :],
                                            op=OP.add)
            # one writer per DMA (hardware limits sync waits per DMA)
            for gl in range(2):
                A_.activation(out=y_sb[:, 0, 512 * gl : 512 * (gl + 1)],
                              in_=fin0[gl][:], func=AF.Copy)
                A_.activation(
                    out=y_sb[:, 0, N1 + 512 * gl : N1 + 512 * (gl + 1)],
                    in_=fin1[gl][:], func=AF.Copy)
            nc.sync.dma_start(out=y_d[:, 1, N1 : 2 * N1],
                              in_=y_sb[:, 1, N1 : 2 * N1])
            nc.sync.dma_start(out=y_d[:, 0, N1 : 2 * N1],
                              in_=y_sb[:, 0, N1 : 2 * N1])
            nc.sync.dma_start(out=y_d[:, 1, 0:N1], in_=y_sb[:, 1, 0:N1])
            nc.sync.dma_start(out=y_d[:, 0, 0:N1], in_=y_sb[:, 0, 0:N1])

    split_waits(nc)
    return nc


def xm_rows(xm, cb, r_start):
    # [128, 8, 64] view of 8 center rows of xm starting at center row
    # r_start: px (r, w) -> col 2 + 66*(1+r_start+r) + w
    base = 2 + 66 * (1 + r_start)
    return xm[:, cb, base : base + 8 * 66].rearrange(
        "p (r w) -> p r w", w=66)[:, :, 0:64]


# ============================ host-side prep ===============================

def _consts():
    perm = [2 * k for k in range(K)] + [2 * k + 1 for k in range(K)]

    selS = np.zeros((128, 2, 32), np.float16)
    for cb in range(2):
        for p in range(128):
            h = (128 * cb + p) // 64
            for j in range(32):
                if j % 4 == h:
                    selS[p, cb, j] = 1.0

    sel16x = np.zeros((16, 32), np.float16)
    sel16y = np.zeros((16, 32), np.float16)
    for j in range(32):
        sel16x[j // 4, j] = 1.0
        sel16y[8 + j // 4, j] = 1.0

    i128 = np.eye(128, dtype=np.float16)
    hs128 = np.zeros((128, 128), np.float16)
    for p in range(128):
        for p2 in range(128):
            if p // 32 == p2 // 32 and p % 4 == p2 % 4:
                hs128[p, p2] = 1.0
    qbg = np.zeros((128, 4, 128), np.float16)
    for g in range(4):
        qbg[32 * g : 32 * g + 32, g, :] = 0.25
    sel48 = np.zeros((80, 3, 16), np.float16)
    for dxi in range(3):
        for o in range(16):
            sel48[32 * dxi + o, dxi, o] = 1.0
    return perm, selS, sel16x, sel16y, i128, hs128, qbg, sel48


def _prep_inputs(x_main, x_aux, offset_w, offset_b, q_w, k_w, out_w):
    perm, selS, sel16x, sel16y, i128, hs128, qbg, sel48 = _consts()

    def wT(wmat):
        r = np.zeros((128, 2, 2, 128), np.float16)
        for cb in range(2):
            for ob in range(2):
                r[:, cb, ob, :] = wmat[128 * ob : 128 * (ob + 1),
                                       128 * cb : 128 * (cb + 1)].T
        return r

    wperm = offset_w[perm]           # [16, C, 3, 3]
    bperm = offset_b[perm].astype(np.float16)
    cwTd = np.zeros((128, 2, 3, 80), np.float16)
    for cb in range(2):
        for dyi in range(3):
            for dxi in range(3):
                cwTd[:, cb, dyi, 32 * dxi : 32 * dxi + 16] = \
                    wperm[:, 128 * cb : 128 * (cb + 1), dyi, dxi].T

    wcat = np.zeros((128, 2960), np.float16)
    wcat[:, 0:512] = wT(q_w).reshape(128, 512)
    wcat[:, 512:1024] = wT(k_w).reshape(128, 512)
    wcat[:, 1024:1536] = wT(out_w).reshape(128, 512)
    wcat[:, 1536:2016] = cwTd.reshape(128, 480)
    wcat[:, 2016:2080] = selS.reshape(128, 64)
    wcat[0:16, 2080:2112] = sel16x
    wcat[0:16, 2112:2144] = sel16y
    wcat[:, 2144:2272] = i128
    wcat[:, 2272:2400] = hs128
    wcat[:, 2400:2912] = qbg.reshape(128, 512)
    wcat[0:80, 2912:2960] = sel48.reshape(80, 48)

    w = np.arange(W, dtype=np.float32)
    xc = [(-0.5 - w), (63.5 - w), (w != W - 1).astype(np.float32),
          -(w != 0).astype(np.float32)]

    in_maps = []
    for core in range(NCORES):
        b, half = core // 2, core % 2
        h0 = ROWS * half
        xm = np.zeros((128, 2, XM_W), np.float16)
        xa = np.zeros((128, 2, XA_W), np.float16)
        for rh in range(HR):
            g = h0 - 1 + rh
            if 0 <= g < H:
                for cb in range(2):
                    xm[:, cb, 2 + 66 * rh : 2 + 66 * rh + 64] = \
                        x_main[b, 128 * cb : 128 * (cb + 1), g, :]
                    xa[:, cb, 64 * rh : 64 * rh + 64] = \
                        x_aux[b, 128 * cb : 128 * (cb + 1), g, :]

        mcat = np.zeros((128, 3841), np.float16)
        for i, v in enumerate(xc):
            mcat[:, 64 * i : 64 * (i + 1)] = v[None, :].astype(np.float16)
        # y consts [128, 512]: p = 32g+j, col = 64*rl + w, row = h0+8g+rl
        gr = np.zeros((128, 512), np.float32)
        for p in range(128):
            gidx = p // 32
            for rl in range(8):
                gr[p, 64 * rl : 64 * rl + 64] = h0 + 8 * gidx + rl
        mcat[:, 256:768] = (-0.5 - gr).astype(np.float16)
        mcat[:, 768:1280] = (63.5 - gr).astype(np.float16)
        mcat[:, 1280:1792] = (gr != H - 1).astype(np.float16)
        mcat[:, 1792:2304] = -(gr != 0).astype(np.float16)
        mcat[0:16, 2304] = bperm
        wv = np.arange(W, dtype=np.float32)
        mxpf = (wv != W - 1).astype(np.float16)
        mxmf = (wv != 0).astype(np.float16)
        mcat[:, 2305:2817] = np.tile(mxpf, 8)[None, :]
        mcat[:, 2817:3329] = np.tile(mxmf, 8)[None, :]
        mcat[:, 3329:3841] = (gr != 0).astype(np.float16)
        in_maps.append(dict(xm=xm, xa=xa, wcat=wcat, mcat=mcat))
    return in_maps


def kernel(**inputs):
    inputs = {k: np.asarray(v, dtype=np.float32) for k, v in inputs.items()}
    if "nc" not in _CACHE:
        _CACHE["nc"] = _build_program()
    nc = _CACHE["nc"]
    in_maps = _prep_inputs(
        inputs["x_main"], inputs["x_aux"], inputs["offset_w"],
        inputs["offset_b"], inputs["q_w"], inputs["k_w"], inputs["out_w"])
    res = run_bass_kernel_spmd(nc, in_maps, list(range(NCORES))).results

    out = np.zeros((B, C, H, W), np.float32)
    for core in range(NCORES):
        b, half = core // 2, core % 2
        y = res[core]["y"].astype(np.float32)  # [128, 2, 2048]
        for ob in range(2):
            out[b, 128 * ob : 128 * (ob + 1),
                ROWS * half : ROWS * (half + 1), :] = \
                y[:, ob, :].reshape(128, ROWS, W)
    return out


# revision 108
# speedup vs baseline: 1.0015x; 1.0015x over previous
"""Trainium2 Bass kernel for nn_MDSFF (deformable-sampling sparse attention).

Math (same restructuring as v1, new layouts/engines):
  - tanh-bounded offsets => bilinear sample == 9-tap stencil, weights
    w_d = relu(1 - |clip(t,lo,hi) - d|) with edge masks; 1x1 convs commute
    with shifts so A = k_w@x_aux and Ao = out_w@x_aux are projected once.
  - sim[k,h] = sum_t TW_t[k] * S_t[h],  S_t[h] = sum_{c in h} q_c * A_t,c.
  - final = sum_t G_t * Ao_t,  G_t = sum_j 0.25 * TW_t[j] * attn[j].

v2 speed structure:
  - fp16 everywhere (rel-err budget 2e-2; measured ~1e-3): DVE TensorTensor
    runs 2x on 2-byte dtypes, tensor_scalar 4x; DMA bytes halve.
  - per-(k,h,pixel) maps in pixel-grouped [128 = 32g+j, 512] layout (g =
    8-row band) -- engine cost is per-column, so 4x fewer columns than the
    v1 [32, 2048] layout.
  - conv: 3 dy-passes accumulate in one PSUM region (dy shift folded into
    the rhs view); only the cheap dx-fold (2 adds) is elementwise.
  - fin accumulated in SBUF fp16 by a DVE/Pool add tree (PSUM has no room).
  - explicit engine assignment tables to balance DVE / Pool / Act.

Sharding: 8 cores = 4 samples x 2 H-halves; each core does its 32 rows in
one full frame (2048 px).
"""

import sys

sys.path.insert(0, "/opt/trn_rl_repo")

import numpy as np

import concourse.bass as bass
import concourse.mybir as mybir
from concourse import tile
from concourse.bass_utils import run_bass_kernel_spmd

# ---------------- problem constants (hardcoded per contract) ----------------
B, C, H, W = 4, 256, 64, 64
K = 8
NCORES = 8
ROWS = 32          # center rows per core
HR = 34            # haloed rows per core
XM_W = 2 + 66 * HR     # padded x_main: col = 2 + 66*rh + w (w-halo via pad)
XA_W = 64 * HR         # x_aux: col = 64*rh + w
AW = 2 + 64 * HR       # A/Ao: col = 1 + 64*rh + w, pad cols 0 and AW-1
N1 = 1024          # px per half-frame
NPX = 2048         # px per core
TAPS = [(dy, dx) for dy in (-1, 0, 1) for dx in (-1, 0, 1)]

F32 = mybir.dt.float32
F16 = mybir.dt.float16
AF = mybir.ActivationFunctionType
OP = mybir.AluOpType

# engine assignment knobs (tuned against the cost model)
M_ENG = ["V", "P", "V", "P", "V", "P", "V", "P", "V"]      # M_t = q*A_t
FV_MODE = ["cA", "dP", "cA", "dP", "dP", "cA", "dP", "dP", "cA"]
#   cV: Pool copies Gb psum->fp16 + DVE mult; cA: Act copy + DVE mult;
#   dP: Pool mult direct from psum
FORDER = [0, 2, 5, 8, 1, 6, 3, 4, 7]   # F-loop tap order
UB = 6             # unified one-bank psum tiles (+2 banks: sim tag)

_CACHE = {}


# ============================ program builder ===============================

def _build_program():
    MAX_WAITS = 1

    SPLIT_OK = {
        "InstDrain", "InstNoOp", "InstMatmult", "InstLdweights",
        "InstTensorTensor", "InstActivation", "InstTensorScalarPtr",
        "InstTensorReduce", "InstCopy", "InstMemSet", "InstMemset",
        "InstReciprocal", "InstTensorTensorReduce", "InstTensorCopy",
        "InstDMACopy",
    }

    def split_waits(nc):
        # walrus rejects >MAX_WAITS semaphore waits per instruction; spill
        # extras onto same-engine nops (engine-FIFO instructions only).
        # DMACopy is included ONLY for the tail y-DMAs: they are the last
        # SP instructions, so stalling the SP sequencer on the spilled wait
        # cannot deadlock (no later SP-pushed DMAs for producers to need).
        f = nc.m.functions[0]
        for bb in f.blocks:
            insts = bb.instructions
            out = []
            changed = False
            for inst in insts:
                si = inst.sync_info
                waits = list(si.on_wait) if si and si.on_wait else []
                if (len(waits) > MAX_WAITS
                        and type(inst).__name__ in SPLIT_OK
                        and all(w.wait_reg is None for w in waits)):
                    changed = True
                    rest, keep = waits[:-MAX_WAITS], waits[-MAX_WAITS:]
                    for i in range(0, len(rest), MAX_WAITS):
                        nop = mybir.InstNoOp(
                            name=f"Wspill_{inst.name}_{i}", ins=[], outs=[])
                        nop.engine = inst.engine
                        nop.sync_info = mybir.SyncInfo(
                            on_wait=rest[i : i + MAX_WAITS], on_update=[])
                        nc.register_instruction(nop)
                        out.append(nop)
                    inst.sync_info = mybir.SyncInfo(
                        on_wait=keep, on_update=list(si.on_update or [])
                    )
                out.append(inst)
            if changed:
                bb.instructions = out

    nc = bass.Bass("TRN2", target_bir_lowering=False, debug=False,
                   num_devices=NCORES)

    dp = nc.dram_tensor
    xm_d = dp("xm", [128, 2, XM_W], F16, kind="ExternalInput")
    xa_d = dp("xa", [128, 2, XA_W], F16, kind="ExternalInput")
    # packed weights: qwT(512) kwT(512) owT(512) cwTd(480) selS(64)
    #                 sel16x(32) sel16y(32) i128(128) hs128(128) qb(512)
    #                 sel48(16)
    wcat_d = dp("wcat", [128, 2960], F16, kind="ExternalInput")
    # packed consts: lox(64) hix(64) mxp(64) negmxm(64)
    #                loy(512) hiy(512) myp(512) negmym(512) bperm(1)
    mcat_d = dp("mcat", [128, 3841], F16, kind="ExternalInput")
    y_d = dp("y", [128, 2, NPX], F16, kind="ExternalOutput")

    V = nc.vector
    P = nc.gpsimd
    A_ = nc.scalar

    def mm(out, lhsT, rhs, start, stop, tp=None):
        nc.tensor.matmul(out=out, lhsT=lhsT, rhs=rhs,
                         start=start, stop=stop, skip_group_check=True,
                         tile_position=tp)

    with tile.TileContext(nc) as tc:
        with (
            nc.allow_low_precision(reason="fp16 data: error budget 2e-2, "
                                   "fp16 chain measured ~1e-3"),
            tc.tile_pool(name="pw", bufs=1) as pw,       # weights/consts
            tc.tile_pool(name="pio", bufs=1) as pio,     # xm, xa, y
            tc.tile_pool(name="pbig", bufs=1) as pbig,   # q, A, Ao, off
            tc.tile_pool(name="pmap", bufs=1) as pmap,   # [128,512] maps
            tc.tile_pool(name="pm", bufs=1) as pm,       # M / Fv / acc
            tc.tile_pool(name="psum", bufs=1, space="PSUM") as psp,
        ):
            xm = pio.tile([128, 2, XM_W], F16, tag="xm")
            xa = pio.tile([128, 2, XA_W], F16, tag="xa")
            wcat = pw.tile([128, 2960], F16, tag="wcat")
            mcat = pw.tile([128, 3841], F16, tag="mcat")
            # all input DMAs on SP (engine-issued DMA time counts as
            # that engine's busy); conv weights slice first so the conv
            # can start at ~2us.
            nc.gpsimd.dma_start(out=wcat[:, 1536:2016],
                                in_=wcat_d[:, 1536:2016])
            for cb in range(2):
                nc.sync.dma_start(out=xm[:, cb, :], in_=xm_d[:, cb, :])
            nc.gpsimd.dma_start(out=mcat[:], in_=mcat_d[:])
            for cb in range(2):
                nc.gpsimd.dma_start(out=xa[:, cb, :], in_=xa_d[:, cb, :])
            nc.sync.dma_start(out=wcat[:, 0:1536], in_=wcat_d[:, 0:1536])
            nc.sync.dma_start(out=wcat[:, 2016:2960],
                              in_=wcat_d[:, 2016:2960])

            def w4(o):  # [128, cb, ob, 128] block at col o
                return wcat[:, o : o + 512].rearrange(
                    "p (cb ob m) -> p cb ob m", cb=2, ob=2)

            qwT, kwT, owT = w4(0), w4(512), w4(1024)
            cwTd = wcat[:, 1536:2016].rearrange(
                "p (cb dy m) -> p cb dy m", cb=2, dy=3)     # m = 32*dxi+o
            selS = wcat[:, 2016:2080].rearrange("p (cb j) -> p cb j", cb=2)
            sel16x = wcat[0:16, 2080:2112]
            sel16y = wcat[0:16, 2112:2144]
            i128 = wcat[:, 2144:2272]
            hs128 = wcat[:, 2272:2400]
            qbg = wcat[:, 2400:2912].rearrange("p (g m) -> p g m", g=4)
            sel48 = wcat[0:80, 2912:2960].rearrange(
                "p (d m) -> p d m", d=3)

            lox = mcat[:, 0:64]
            hix = mcat[:, 64:128]
            mxp = mcat[:, 128:192]
            negmxm = mcat[:, 192:256]
            loy = mcat[:, 256:768]
            hiy = mcat[:, 768:1280]
            myp = mcat[:, 1280:1792]
            negmym = mcat[:, 1792:2304]
            bperm = mcat[0:16, 2304:2305]
            mxp512 = mcat[:, 2305:2817]
            mxm512 = mcat[:, 2817:3329]
            mym512 = mcat[:, 3329:3841]

            def xb(t):  # broadcast [128,64] x-const over 8 rows -> [128,8,64]
                return t[:, None, :].broadcast_to([128, 8, 64])

            # ---------------- offset conv (3 dy-passes) + fold + tanh -------
            # D[16*dxi+o, 64*r + wc] = sum_{c,dy} w[o,c,dy,dxi]
            #                                    * xm[c, r+dy, wc-1]
            # rhs flat view: 66-strided rows, start col = 2+66*(1+r0+dy)-1
            # D[16*dxi+o, 66*rr + wc] = sum_{c,dy} w[o,c,dy,dxi]
            #   * xm[c, r0+rr+dy, wc-1];  off tap dxi reads wc = w + dxi.
            off = pbig.tile([16, NPX], F16, tag="off")
            dsbs = {}

            def conv_q(qt):
                r0 = 8 * qt
                dsb = pm.tile([80, 528], F16, tag="dsb", bufs=3,
                              name=f"dsb{qt}")
                for c0, sz in ((0, 512), (512, 16)):
                    dps = psp.tile([128, 512], F32, tag="u", bufs=UB,
                                   name=f"dps{qt}{c0}")
                    nmm = 0
                    for dyi in range(3):
                        for cb in range(2):
                            base = 2 + 66 * (r0 + dyi) - 1
                            mm(dps[0:80, 0:sz],
                               cwTd[:, cb, dyi, :],
                               xm[:, cb, base + c0 : base + c0 + sz],
                               start=(nmm == 0), stop=(nmm == 5))
                            nmm += 1
                    V.tensor_scalar(out=dsb[:, c0 : c0 + sz],
                                    in0=dps[0:80, 0:sz], scalar1=1.0,
                                    scalar2=None, op0=OP.mult)
                dsbs[qt] = dsb

            def fold_q(qt):
                dv = dsbs[qt].rearrange("p (r wc) -> p r wc", wc=66)
                fps = psp.tile([128, 512], F32, tag="u", bufs=UB,
                               name=f"fold{qt}")
                for dxi in range(3):
                    mm(fps[0:16, :], sel48[:, dxi, :],
                       dv[:, 0:8, dxi : dxi + 64],
                       start=(dxi == 0), stop=(dxi == 2))
                A_.activation(out=off[:, 512 * qt : 512 * (qt + 1)],
                              in_=fps[0:16, :], func=AF.Tanh, bias=bperm,
                              scale=1.0)

            conv_q(0); conv_q(1); fold_q(0); conv_q(2)
            fold_q(1); conv_q(3); fold_q(2); fold_q(3)
            # (q-proj mms interleave naturally below; PE stays fed)

            # ---------------- q / A / Ao projections ----------------
            # ---------------- tap-weight maps (grouped [128,512]) ----------
            # partition p = 32g+j (j = 4k+h); col = 64*(r%8)+w, g = r//8.
            # Scheduled right after the conv so the DVE/Pool chains overlap
            # the q/A projections on PE; x-chain on DVE, y-chain on Pool.
            txp = psp.tile([128, 512], F32, tag="u", bufs=UB, name="txp")
            typ = psp.tile([128, 512], F32, tag="u", bufs=UB, name="typ")
            for g in range(4):
                mm(txp[32 * g : 32 * g + 32, :], sel16x,
                   off[:, 512 * g : 512 * (g + 1)], start=True, stop=True,
                   tp=(0, 32 * g))
            for g in range(4):
                mm(typ[32 * g : 32 * g + 32, :], sel16y,
                   off[:, 512 * g : 512 * (g + 1)], start=True, stop=True,
                   tp=(0, 32 * g))

            def gv(t):  # [128, 512] -> [128, 8, 64]
                return t.rearrange("p (r w) -> p r w", w=64)

            tcx = pmap.tile([128, 512], F16, tag="mw", bufs=4, name="tcx")
            V.tensor_tensor(out=gv(tcx), in0=gv(txp), in1=xb(lox), op=OP.max)
            V.tensor_tensor(out=gv(tcx), in0=gv(tcx), in1=xb(hix), op=OP.min)
            tcy = pmap.tile([128, 512], F16, tag="mw", bufs=4, name="tcy")
            V.tensor_tensor(out=tcy[:], in0=typ[:], in1=loy, op=OP.max)
            V.tensor_tensor(out=tcy[:], in0=tcy[:], in1=hiy, op=OP.min)

            # weight maps on DVE/Act only: Pool's software TensorTensor
            # implements just mult/add, and cannot touch PSUM.
            wx, wy = {}, {}
            for ax, (tc_, mkp, mkm) in enumerate(
                    ((tcx, mxp512, mxm512), (tcy, myp, mym512))):
                wd = {}
                for d in (1, -1, 0):
                    wd[d] = pmap.tile([128, 512], F16, tag=f"w{ax}{d}",
                                      name=f"w{ax}_{d}")
                V.scalar_tensor_tensor(out=wd[1][:], in0=tc_[:], scalar=0.0,
                                       in1=mkp, op0=OP.max, op1=OP.mult)
                A_.activation(out=wd[-1][:], in_=tc_[:], func=AF.Relu,
                              scale=-1.0)
                P.tensor_tensor(out=wd[-1][:], in0=wd[-1][:], in1=mkm,
                                op=OP.mult)
                A_.activation(out=wd[0][:], in_=tc_[:], func=AF.Abs)
                V.tensor_scalar(out=wd[0][:], in0=wd[0][:], scalar1=-1.0,
                                scalar2=1.0, op0=OP.mult, op1=OP.add)
                (wx, wy)[ax].update(wd)

            TW = []
            for t, (dy, dx) in enumerate(TAPS):
                tw = pmap.tile([128, 512], F16, tag=f"tw{t}", name=f"TW{t}")
                P.tensor_tensor(out=tw[:], in0=wy[dy][:], in1=wx[dx][:],
                                op=OP.mult)
                TW.append(tw)

            # ---------------- q / A projections ----------------
            # PSUM->SBUF copies rotate across Act/Pool/DVE so no single
            # engine serializes the A assembly (M_t needs the whole of A).
            cpcnt = [0]

            def copy_ps(dst_view, ps_view):
                i = cpcnt[0] % 3
                cpcnt[0] += 1
                if i == 1:
                    V.tensor_scalar(out=dst_view, in0=ps_view, scalar1=1.0,
                                    scalar2=None, op0=OP.mult)
                else:
                    A_.activation(out=dst_view, in_=ps_view, func=AF.Copy)

            q = pbig.tile([128, 2, NPX], F16, tag="q")
            AT = pbig.tile([128, 2, AW], F16, tag="A")
            AoT = pbig.tile([128, 2, AW], F16, tag="Ao")
            for dst in (AT, AoT):
                V.memset(dst[:, :, 0:1], 0.0)
                V.memset(dst[:, :, AW - 1 : AW], 0.0)

            def q_chunk(ob, hf, c0):
                ps = psp.tile([128, 512], F32, tag="u", bufs=UB,
                              name=f"psq{ob}{hf}{c0}")
                for cb in range(2):
                    mm(ps[:], qwT[:, cb, ob, :],
                       xm_rows(xm, cb, 16 * hf + c0 // 64),
                       start=(cb == 0), stop=(cb == 1))
                copy_ps(q[:, ob, N1 * hf + c0 : N1 * hf + c0 + 512], ps[:])

            def proj_chunk(dst, wT, ob, p0, di):
                sz = min(512, 2176 - p0)
                ps = psp.tile([128, 512], F32, tag="u", bufs=UB,
                              name=f"psP{di}{ob}{p0}")
                for cb in range(2):
                    mm(ps[:, 0:sz], wT[:, cb, ob, :],
                       xa[:, cb, p0 : p0 + sz],
                       start=(cb == 0), stop=(cb == 1))
                copy_ps(dst[:, ob, 1 + p0 : 1 + p0 + sz], ps[:, 0:sz])

            # chunks needed by M's first half (q cols 0:1024, A cols
            # 0:~1090) are emitted first so the S-loop can start early
            mixed = []
            for ob in range(2):
                for c0 in (0, 512):
                    mixed.append(("q", ob, c0))
            for p0 in (0, 512, 1024):
                for ob in range(2):
                    mixed.append(("A", ob, p0))
            for ob in range(2):
                for c0 in (0, 512):
                    mixed.append(("q", ob, 1024 + c0))
            for p0 in (1536, 2048):
                for ob in range(2):
                    mixed.append(("A", ob, p0))
            for kind, ob, p0 in mixed:
                if kind == "q":
                    q_chunk(ob, p0 // 1024, p0 % 1024)
                else:
                    proj_chunk(AT, kwT, ob, p0, 0)
            # Ao chunks are interleaved into the S-loop below: they fill PE
            # gaps while M tiles are being produced, and Ao is only needed
            # by the F phase.
            ao_chunks = [(ob, p0) for ob in range(2)
                         for p0 in range(0, 2176, 512)]

            # ---------------- S maps + sim + softmax ----------------
            # sim accumulated on DVE (fp32 SBUF adds) to keep the sim chain
            # off the PE, which is the S-loop bottleneck.
            sim_sb = pmap.tile([128, 512], F16, tag="simb", name="simb")
            # NOTE: "sim" psum tag tiles are reused as fin1 in the F phase
            Ps = []
            for t, (dy, dx) in enumerate(TAPS):
                o_t = 65 + 64 * dy + dx
                M = pm.tile([128, 2, NPX], F16, tag="M", bufs=4,
                            name=f"M{t}")
                eng = V if M_ENG[t] == "V" else P
                nmh = 4 if t == 0 else 2
                for mh in range(nmh):
                    w = NPX // nmh
                    me = (V, P)[mh % 2] if t == 0 else eng
                    me.tensor_tensor(
                        out=M[:, :, w * mh : w * (mh + 1)],
                        in0=q[:, :, w * mh : w * (mh + 1)],
                        in1=AT[:, :, o_t + w * mh : o_t + w * mh + w],
                        op=OP.mult)
                s_ps = psp.tile([128, 512], F32, tag="u", bufs=UB,
                                name=f"sps{t}")
                for g in range(4):
                    for cb in range(2):
                        mm(s_ps[32 * g : 32 * g + 32, :], selS[:, cb, :],
                           M[:, cb, 512 * g : 512 * (g + 1)],
                           start=(cb == 0), stop=(cb == 1), tp=(0, 32 * g))
                P_t = pmap.tile([128, 512], F16, tag="sp", bufs=3,
                                name=f"P{t}")
                V.tensor_tensor(out=P_t[:], in0=s_ps[:], in1=TW[t][:],
                                op=OP.mult)
                if t == 1:
                    P.tensor_tensor(out=sim_sb[:], in0=Ps[0][:], in1=P_t[:],
                                    op=OP.add)
                elif t > 1:
                    P.tensor_tensor(out=sim_sb[:], in0=sim_sb[:], in1=P_t[:],
                                    op=OP.add)
                Ps.append(P_t if t == 0 else None)
                while ao_chunks and len(ao_chunks) > max(4, 10 - 2 * (t + 1)):
                    ob, p0 = ao_chunks.pop(0)
                    proj_chunk(AoT, owT, ob, p0, 1)

            for ob, p0 in ao_chunks:
                proj_chunk(AoT, owT, ob, p0, 1)
            ao_chunks = []

            # softmax chain in column halves: halves the serial latency
            # of the S->F barrier (exp -> denom -> recip -> normalize)
            E = pmap.tile([128, 512], F16, tag="smE", name="E")
            d_ps = psp.tile([128, 512], F32, tag="u", bufs=UB, name="dps")
            Rr = pmap.tile([128, 512], F16, tag="smR", name="R")
            Ff = pmap.tile([128, 512], F16, tag="smF", name="F")
            for sh in range(2):
                sl = slice(256 * sh, 256 * (sh + 1))
                A_.activation(out=E[:, sl], in_=sim_sb[:, sl], func=AF.Exp,
                              bias=0.0, scale=0.125)
                mm(d_ps[:, sl], hs128, E[:, sl], start=True, stop=True)
                V.reciprocal(out=Rr[:, sl], in_=d_ps[:, sl])
                V.tensor_tensor(out=Ff[:, sl], in0=E[:, sl], in1=Rr[:, sl],
                                op=OP.mult)

            # ---------------- final combine ----------------
            # Q_t is half-independent: compute the 9 maps once.
            Qs = []
            for t in range(9):
                Q_t = pmap.tile([128, 512], F16, tag="qg", bufs=9,
                                name=f"Q{t}")
                P.tensor_tensor(out=Q_t[:], in0=TW[t][:], in1=Ff[:],
                                op=OP.mult)
                Qs.append(Q_t)

            y_sb = pio.tile([128, 2, NPX], F16, tag="y")
            # Both half-frames interleaved per tap (independent chains hide
            # per-hop latency; only hf0-ob0 pins psum).  Accumulation:
            # hf0-ob0 on PE psum, hf0-ob1 and hf1(both ob) as running
            # in-place adds paced by Fv arrival (tail depth = 1 add).
            FV_MODES = ["DV", "AP", "AV", "AP", "AV", "AP", "AV", "AP", "DV"]
            ACC_ENG = [V, P, V, P, V, P, V, V]
            fin0 = [psp.tile([128, 512], F32, tag="u", bufs=UB,
                             name=f"fin0{gl}") for gl in range(2)]
            fin1 = [psp.tile([128, 512], F32, tag="sim", bufs=2,
                             name=f"fin1{gl}") for gl in range(2)]
            acc0 = pm.tile([128, N1], F16, tag="acc", bufs=2, name="acc0")
            acc1 = pm.tile([128, N1], F16, tag="acc", bufs=2,
                           name="acc1")
            prev = [None, None]
            for i, t in enumerate(FORDER):
                dy, dx = TAPS[t]
                for hf in range(2):
                    o_t = 65 + 64 * dy + dx + N1 * hf
                    Fv = pm.tile([128, 2, N1], F16, tag="Fv", bufs=5,
                                 name=f"Fv{hf}{t}")
                    for gl in range(2):
                        g = 2 * hf + gl
                        gb_ps = psp.tile([128, 512], F32, tag="u", bufs=UB,
                                         name=f"gb{hf}{t}{gl}")
                        mm(gb_ps[:], qbg[:, g, :], Qs[t][:],
                           start=True, stop=True)
                        ov = o_t + 512 * gl
                        md = FV_MODES[i]
                        fvv = Fv[:, :, 512 * gl : 512 * (gl + 1)]
                        if md == "DV":
                            V.tensor_tensor(
                                out=fvv,
                                in0=gb_ps[:, None, :].broadcast_to(
                                    [128, 2, 512]),
                                in1=AoT[:, :, ov : ov + 512], op=OP.mult)
                        else:
                            gsb = pm.tile([128, 512], F16, tag="gsb",
                                          bufs=4, name=f"gsb{hf}{t}{gl}")
                            if md == "VP":
                                V.tensor_scalar(out=gsb[:], in0=gb_ps[:],
                                                scalar1=1.0, scalar2=None,
                                                op0=OP.mult)
                            else:
                                A_.activation(out=gsb[:], in_=gb_ps[:],
                                              func=AF.Copy)
                            (P if md in ("AP", "VP") else V).tensor_tensor(
                                out=fvv,
                                in0=gsb[:, None, :].broadcast_to(
                                    [128, 2, 512]),
                                in1=AoT[:, :, ov : ov + 512], op=OP.mult)
                    if hf == 0:
                        for gl in range(2):
                            mm(fin0[gl][:], i128,
                               Fv[:, 0, 512 * gl : 512 * (gl + 1)],
                               start=(i == 0), stop=(i == 8))
                        # ob1 running accumulation
                        if i == 0:
                            prev[0] = Fv
                        elif i == 1:
                            V.tensor_tensor(out=acc0[:],
                                            in0=prev[0][:, 1, :],
                                            in1=Fv[:, 1, :], op=OP.add)
                        elif i < 8:
                            ACC_ENG[i - 1].tensor_tensor(
                                out=acc0[:], in0=acc0[:], in1=Fv[:, 1, :],
                                op=OP.add)
                        else:
                            V.tensor_tensor(out=y_sb[:, 1, 0:N1],
                                            in0=acc0[:], in1=Fv[:, 1, :],
                                            op=OP.add)
                    else:
                        for gl in range(2):
                            mm(fin1[gl][:], i128,
                               Fv[:, 0, 512 * gl : 512 * (gl + 1)],
                               start=(i == 0), stop=(i == 8))
                        if i == 0:
                            prev[1] = Fv
                        elif i == 1:
                            V.tensor_tensor(out=acc1[:],
                                            in0=prev[1][:, 1, :],
                                            in1=Fv[:, 1, :], op=OP.add)
                        elif i < 8:
                            ACC_ENG[8 - i].tensor_tensor(
                                out=acc1[:], in0=acc1[:], in1=Fv[:, 1, :],
                                op=OP.add)
                        else:
                            V.tensor_tensor(out=y_sb[:, 1, N1 : 2 * N1],
                                            in0=acc1[:], in1=Fv[:, 1, :],
                                            op=OP.add)
            # one writer per DMA (hardware limits sync waits per DMA)
            for gl in range(2):
                A_.activation(out=y_sb[:, 0, 512 * gl : 512 * (gl + 1)],
                              in_=fin0[gl][:], func=AF.Copy)
                A_.activation(
                    out=y_sb[:, 0, N1 + 512 * gl : N1 + 512 * (gl + 1)],
                    in_=fin1[gl][:], func=AF.Copy)
            # two DMA queues in parallel for the output tail
            nc.sync.dma_start(out=y_d[:, 1, N1 : 2 * N1],
                              in_=y_sb[:, 1, N1 : 2 * N1])
            nc.gpsimd.dma_start(out=y_d[:, 0, N1 : 2 * N1],
                                in_=y_sb[:, 0, N1 : 2 * N1])
            nc.sync.dma_start(out=y_d[:, 1, 0:N1], in_=y_sb[:, 1, 0:N1])
            nc.gpsimd.dma_start(out=y_d[:, 0, 0:N1], in_=y_sb[:, 0, 0:N1])

    split_waits(nc)
    return nc


def xm_rows(xm, cb, r_start):
    # [128, 8, 64] view of 8 center rows of xm starting at center row
    # r_start: px (r, w) -> col 2 + 66*(1+r_start+r) + w
    base = 2 + 66 * (1 + r_start)
    return xm[:, cb, base : base + 8 * 66].rearrange(
        "p (r w) -> p r w", w=66)[:, :, 0:64]


# ============================ host-side prep ===============================

def _consts():
    perm = [2 * k for k in range(K)] + [2 * k + 1 for k in range(K)]

    selS = np.zeros((128, 2, 32), np.float16)
    for cb in range(2):
        for p in range(128):
            h = (128 * cb + p) // 64
            for j in range(32):
                if j % 4 == h:
                    selS[p, cb, j] = 1.0

    sel16x = np.zeros((16, 32), np.float16)
    sel16y = np.zeros((16, 32), np.float16)
    for j in range(32):
        sel16x[j // 4, j] = 1.0
        sel16y[8 + j // 4, j] = 1.0

    i128 = np.eye(128, dtype=np.float16)
    hs128 = np.zeros((128, 128), np.float16)
    for p in range(128):
        for p2 in range(128):
            if p // 32 == p2 // 32 and p % 4 == p2 % 4:
                hs128[p, p2] = 1.0
    qbg = np.zeros((128, 4, 128), np.float16)
    for g in range(4):
        qbg[32 * g : 32 * g + 32, g, :] = 0.25
    sel48 = np.zeros((80, 3, 16), np.float16)
    for dxi in range(3):
        for o in range(16):
            sel48[32 * dxi + o, dxi, o] = 1.0
    return perm, selS, sel16x, sel16y, i128, hs128, qbg, sel48


def _prep_inputs(x_main, x_aux, offset_w, offset_b, q_w, k_w, out_w):
    perm, selS, sel16x, sel16y, i128, hs128, qbg, sel48 = _consts()

    def wT(wmat):
        r = np.zeros((128, 2, 2, 128), np.float16)
        for cb in range(2):
            for ob in range(2):
                r[:, cb, ob, :] = wmat[128 * ob : 128 * (ob + 1),
                                       128 * cb : 128 * (cb + 1)].T
        return r

    wperm = offset_w[perm]           # [16, C, 3, 3]
    bperm = offset_b[perm].astype(np.float16)
    cwTd = np.zeros((128, 2, 3, 80), np.float16)
    for cb in range(2):
        for dyi in range(3):
            for dxi in range(3):
                cwTd[:, cb, dyi, 32 * dxi : 32 * dxi + 16] = \
                    wperm[:, 128 * cb : 128 * (cb + 1), dyi, dxi].T

    wcat = np.zeros((128, 2960), np.float16)
    wcat[:, 0:512] = wT(q_w).reshape(128, 512)
    wcat[:, 512:1024] = wT(k_w).reshape(128, 512)
    wcat[:, 1024:1536] = wT(out_w).reshape(128, 512)
    wcat[:, 1536:2016] = cwTd.reshape(128, 480)
    wcat[:, 2016:2080] = selS.reshape(128, 64)
    wcat[0:16, 2080:2112] = sel16x
    wcat[0:16, 2112:2144] = sel16y
    wcat[:, 2144:2272] = i128
    wcat[:, 2272:2400] = hs128
    wcat[:, 2400:2912] = qbg.reshape(128, 512)
    wcat[0:80, 2912:2960] = sel48.reshape(80, 48)

    w = np.arange(W, dtype=np.float32)
    xc = [(-0.5 - w), (63.5 - w), (w != W - 1).astype(np.float32),
          -(w != 0).astype(np.float32)]

    in_maps = []
    for core in range(NCORES):
        b, half = core // 2, core % 2
        h0 = ROWS * half
        xm = np.zeros((128, 2, XM_W), np.float16)
        xa = np.zeros((128, 2, XA_W), np.float16)
        for rh in range(HR):
            g = h0 - 1 + rh
            if 0 <= g < H:
                for cb in range(2):
                    xm[:, cb, 2 + 66 * rh : 2 + 66 * rh + 64] = \
                        x_main[b, 128 * cb : 128 * (cb + 1), g, :]
                    xa[:, cb, 64 * rh : 64 * rh + 64] = \
                        x_aux[b, 128 * cb : 128 * (cb + 1), g, :]

        mcat = np.zeros((128, 3841), np.float16)
        for i, v in enumerate(xc):
            mcat[:, 64 * i : 64 * (i + 1)] = v[None, :].astype(np.float16)
        # y consts [128, 512]: p = 32g+j, col = 64*rl + w, row = h0+8g+rl
        gr = np.zeros((128, 512), np.float32)
        for p in range(128):
            gidx = p // 32
            for rl in range(8):
                gr[p, 64 * rl : 64 * rl + 64] = h0 + 8 * gidx + rl
        mcat[:, 256:768] = (-0.5 - gr).astype(np.float16)
        mcat[:, 768:1280] = (63.5 - gr).astype(np.float16)
        mcat[:, 1280:1792] = (gr != H - 1).astype(np.float16)
        mcat[:, 1792:2304] = -(gr != 0).astype(np.float16)
        mcat[0:16, 2304] = bperm
        wv = np.arange(W, dtype=np.float32)
        mxpf = (wv != W - 1).astype(np.float16)
        mxmf = (wv != 0).astype(np.float16)
        mcat[:, 2305:2817] = np.tile(mxpf, 8)[None, :]
        mcat[:, 2817:3329] = np.tile(mxmf, 8)[None, :]
        mcat[:, 3329:3841] = (gr != 0).astype(np.float16)
        in_maps.append(dict(xm=xm, xa=xa, wcat=wcat, mcat=mcat))
    return in_maps


def kernel(**inputs):
    inputs = {k: np.asarray(v, dtype=np.float32) for k, v in inputs.items()}
    if "nc" not in _CACHE:
        _CACHE["nc"] = _build_program()
    nc = _CACHE["nc"]
    in_maps = _prep_inputs(
        inputs["x_main"], inputs["x_aux"], inputs["offset_w"],
        inputs["offset_b"], inputs["q_w"], inputs["k_w"], inputs["out_w"])
    res = run_bass_kernel_spmd(nc, in_maps, list(range(NCORES))).results

    out = np.zeros((B, C, H, W), np.float32)
    for core in range(NCORES):
        b, half = core // 2, core % 2
        y = res[core]["y"].astype(np.float32)  # [128, 2, 2048]
        for ob in range(2):
            out[b, 128 * ob : 128 * (ob + 1),
                ROWS * half : ROWS * (half + 1), :] = \
                y[:, ob, :].reshape(128, ROWS, W)
    return out
